# revision 17
# baseline (speedup 1.0000x reference)
"""MultiHeadAttn (post-LN, key-padding mask) Trainium2 Bass kernel, 8 cores.

Problem: h [S=2048, B=4, D=1024] f32; 16 heads x 64; key-padding mask [S, B];
out = LayerNorm(h + MHA(h)).

Sharding: core c handles batch b = c//2 and query half qh = c%2 (1024 query
rows), with all 16 heads and the full 2048-key context for that batch.
KV projections are recomputed by both cores of a batch pair (no collectives).

Per-core device pipeline (all matmuls bf16, fp32 accumulation in PSUM):
  - K^T/Q^T proj: stationary Wk/Wq column tiles, moving h^T -> [e, t] layout.
  - V proj: stationary h^T tiles, moving Wv -> natural [t, e] layout, stored
    with an appended ones column per head (gives softmax denominators via PV).
  - Attention per head pair: scores^T [j,i] via row-paired matmuls (two heads
    in row strips 0-63 / 64-127 of the PE array), exp via ScalarE with the
    key-padding bias as a per-partition bias and 1/sqrt(dh) as the scale,
    PV with ones-augmented V (M=65; row 64 accumulates the denominator),
    then normalize via reciprocal + partition broadcast + multiply.
  - Output proj: stationary attn_vec^T tiles, moving Wo; residual add + LN
    (bn_stats/bn_aggr) fused on DVE/ScalarE.
Next head pair's K/Q projections are interleaved into the attention loop
(borrowing scores-pool PSUM slots) so the PE stays busy under the ACT-bound
softmax stream.
"""
import numpy as np
import ml_dtypes

N_HEAD, D_MODEL, D_HEAD = 16, 1024, 64
SEQ, BSZ = 2048, 4
QLEN = SEQ // 2
SCALE = 1.0 / D_HEAD ** 0.5
LN_EPS = 1e-5
P = 128
NSL = 512                   # matmul moving-operand slab (one PSUM bank fp32)
CT = D_MODEL // P           # 8 contraction tiles
ET = D_MODEL // P           # 8 e-tiles (2 heads each)
JT = SEQ // P               # 16 key tiles
JS = SEQ // NSL             # 4 key slabs
IS = QLEN // NSL            # 2 query slabs
TQ = QLEN // P              # 8 query-row tiles
HP = N_HEAD // 2            # 8 head pairs

_CACHE = {}


def _build():
    from contextlib import ExitStack
    import concourse.bass as bass
    import concourse.mybir as mybir
    import concourse.tile as tile
    from concourse import bacc

    dt = mybir.dt
    f32, bf16 = dt.float32, dt.bfloat16
    AF = mybir.ActivationFunctionType
    ALU = mybir.AluOpType

    nc = bacc.Bacc(None, target_bir_lowering=False)

    hT = nc.dram_tensor("hT", [D_MODEL, SEQ], bf16, kind="ExternalInput")
    hTq = nc.dram_tensor("hTq", [D_MODEL, QLEN], bf16, kind="ExternalInput")
    hq = nc.dram_tensor("hq", [QLEN, D_MODEL], f32, kind="ExternalInput")
    wq = nc.dram_tensor("wq", [D_MODEL, D_MODEL], bf16, kind="ExternalInput")
    wk = nc.dram_tensor("wk", [D_MODEL, D_MODEL], bf16, kind="ExternalInput")
    wv = nc.dram_tensor("wv", [D_MODEL, D_MODEL], bf16, kind="ExternalInput")
    wo = nc.dram_tensor("wo", [D_MODEL, D_MODEL], bf16, kind="ExternalInput")
    mb = nc.dram_tensor("mb", [SEQ], f32, kind="ExternalInput")
    gam = nc.dram_tensor("gam", [D_MODEL], f32, kind="ExternalInput")
    bet = nc.dram_tensor("bet", [D_MODEL], f32, kind="ExternalInput")
    out = nc.dram_tensor("out", [QLEN, D_MODEL], f32, kind="ExternalOutput")

    with tile.TileContext(nc) as tc, ExitStack() as ctx:
        persist = ctx.enter_context(tc.tile_pool(name="persist", bufs=1))

        kt_sb = [persist.tile([P, SEQ], bf16, name=f"kt{e}") for e in range(ET)]
        qt_sb = [persist.tile([P, QLEN], bf16, name=f"qt{e}") for e in range(ET)]
        v_sb = [persist.tile([P, N_HEAD, D_HEAD], bf16, name=f"v{t}")
                for t in range(JT)]
        ones64 = persist.tile([P, 64], bf16, name="ones64")
        avt_sb = [persist.tile([P, QLEN], bf16, name=f"avt{e}") for e in range(ET)]
        mask_sb = persist.tile([P, JT], f32, name="mask")
        eps_sb = persist.tile([P, 1], f32, name="eps")

        nc.vector.memset(eps_sb, LN_EPS)
        nc.vector.memset(ones64, 1.0)

        nc.gpsimd.dma_start(out=mask_sb,
                            in_=bass.AP(tensor=mb, offset=0, ap=[[1, P], [P, JT]]))

        # ---- phase-3 weights: load early into the region wvp freed ---------
        w3p = ctx.enter_context(tc.tile_pool(name="w3p", bufs=1))
        wo_sb = [w3p.tile([P, D_MODEL], bf16, name=f"wo{c}") for c in range(CT)]
        gam_sb = w3p.tile([P, D_MODEL], f32, name="gamr")
        bet_sb = w3p.tile([P, D_MODEL], f32, name="betr")

        # ---- phase 1 scope: h^T residency + streamed W columns --------------
        ph1_ctx = ExitStack()
        ph1 = ph1_ctx.enter_context(tc.tile_pool(name="ph1", bufs=1))
        ht_sb = [ph1.tile([P, SEQ], bf16, name=f"ht{c}") for c in range(CT)]
        htq_sb = [ph1.tile([P, QLEN], bf16, name=f"htq{c}") for c in range(CT)]

        wcol = ph1_ctx.enter_context(tc.tile_pool(name="wcol", bufs=3))

        def load_wcol(w, e, tag):
            wc = wcol.tile([P, CT, P], bf16, tag=tag, name=f"{tag}{e}")
            nc.sync.dma_start(
                out=wc,
                in_=w[:, e * P:(e + 1) * P].rearrange("(ct p) e -> p ct e", p=P))
            return wc

        # startup DMA priority: the first K-proj matmul needs wkc(0) + ht
        # tiles, so those go first on the sync queue; htq (needed later, for
        # Q-proj) goes via gpsimd SWDGE in parallel.
        wc0 = load_wcol(wk, 0, "wkc")
        for c in range(CT):
            eng = nc.sync if c % 2 == 0 else nc.scalar
            eng.dma_start(out=ht_sb[c], in_=hT[c * P:(c + 1) * P, :])
        for c in range(CT):
            nc.gpsimd.dma_start(out=htq_sb[c], in_=hTq[c * P:(c + 1) * P, :])

        def kq_group(ps_ap, wc, moving, sl):
            """8 accumulating matmuls: one K/Q-proj output group into psum."""
            for c in range(CT):
                nc.tensor.matmul(ps_ap, wc[:, c, :],
                                 moving[c][:, sl * NSL:(sl + 1) * NSL],
                                 start=(c == 0), stop=(c == CT - 1))

        # prephase: K(0), Q(0), V (own pools, closed before attention)
        with tc.tile_pool(name="wvp", bufs=1) as wvp, \
             tc.tile_pool(name="psA", bufs=6, space="PSUM") as psA:
            wv_sb = [wvp.tile([P, D_MODEL], bf16, name=f"wv{c}") for c in range(CT)]
            for c in range(CT):
                nc.scalar.dma_start(out=wv_sb[c], in_=wv[c * P:(c + 1) * P, :])
            wc = wc0
            for j in range(JS):
                ps = psA.tile([P, NSL], f32, tag="psa", name=f"psk0_{j}")
                kq_group(ps, wc, ht_sb, j)
                nc.vector.tensor_copy(kt_sb[0][:, j * NSL:(j + 1) * NSL], ps)
            wc = load_wcol(wq, 0, "wqc")
            for i in range(IS):
                ps = psA.tile([P, NSL], f32, tag="psa", name=f"psq0_{i}")
                kq_group(ps, wc, htq_sb, i)
                nc.vector.tensor_copy(qt_sb[0][:, i * NSL:(i + 1) * NSL], ps)
            # V projection: stationary h^T tiles, moving Wv slabs
            for t in range(JT):
                for es in range(2):
                    ps = psA.tile([P, NSL], f32, tag="psa", name=f"psv{t}_{es}")
                    for c in range(CT):
                        nc.tensor.matmul(ps, ht_sb[c][:, t * P:(t + 1) * P],
                                         wv_sb[c][:, es * NSL:(es + 1) * NSL],
                                         start=(c == 0), stop=(c == CT - 1))
                    nc.vector.tensor_copy(
                        v_sb[t][:, es * 8:(es + 1) * 8, :],
                        ps[:, :].rearrange("p (h d) -> p h d", d=D_HEAD))

        def emit_pv(nc, v_sb, ones64, av, den, hp, j, pts):
            first, last = (j == 0), (j == JT - 1)
            for i in range(IS):
                sl = slice(i * NSL, (i + 1) * NSL)
                nc.tensor.matmul(av[i][0:64, :], v_sb[j][:, hp * 2, :],
                                 pts[0][:, sl], start=first, stop=last,
                                 tile_position=(0, 0))
                nc.tensor.matmul(av[i][64:P, :], v_sb[j][:, hp * 2 + 1, :],
                                 pts[1][:, sl], start=first, stop=last,
                                 tile_position=(0, 64), skip_group_check=True)
                nc.tensor.matmul(den[i][0:64, :], ones64, pts[0][:, sl],
                                 start=first, stop=last,
                                 tile_position=(0, 0), skip_group_check=True)
                nc.tensor.matmul(den[i][64:P, :], ones64, pts[1][:, sl],
                                 start=first, stop=last,
                                 tile_position=(0, 64), skip_group_check=True)

        for c in range(CT):
            nc.scalar.dma_start(out=wo_sb[c], in_=wo[c * P:(c + 1) * P, :])
        nc.gpsimd.dma_start(out=gam_sb,
                            in_=bass.AP(tensor=gam, offset=0, ap=[[0, P], [1, D_MODEL]]))
        nc.gpsimd.dma_start(out=bet_sb,
                            in_=bass.AP(tensor=bet, offset=0, ap=[[0, P], [1, D_MODEL]]))

        # ---- attention ------------------------------------------------------
        attn_ctx = ExitStack()
        scp = attn_ctx.enter_context(tc.tile_pool(name="scp", bufs=2, space="PSUM"))
        avp = attn_ctx.enter_context(tc.tile_pool(name="avp", bufs=2, space="PSUM"))
        ptp = attn_ctx.enter_context(tc.tile_pool(name="ptp", bufs=6))
        nrm = attn_ctx.enter_context(tc.tile_pool(name="nrm", bufs=2))

        for hp in range(HP):
            av = [avp.tile([P, NSL], f32, tag="av", name=f"av{hp}_{i}")
                  for i in range(IS)]
            den = [avp.tile([P, NSL], f32, tag="den", name=f"den{hp}_{i}")
                   for i in range(IS)]
            # interleaved projection work for the NEXT head pair, borrowing
            # scores-pool psum slots: (emit_at_j, which, slab)
            proj_work = {4: ("k", 0), 8: ("k", 2), 12: ("q", 0)} if hp + 1 < HP else {}
            prev_pt = None
            wc_k = None

            for j in range(JT):
                cur_pt = []
                for hb in range(2):
                    h = hp * 2 + hb
                    base = hb * 64
                    sc = scp.tile([P, QLEN], f32, tag="sc", name=f"sc{hp}_{j}_{hb}")
                    for i in range(IS):
                        nc.tensor.matmul(
                            sc[:, i * NSL:(i + 1) * NSL],
                            kt_sb[hp][base:base + 64, j * P:(j + 1) * P],
                            qt_sb[hp][base:base + 64, i * NSL:(i + 1) * NSL],
                            start=True, stop=True, tile_position=(base, 0))
                    pt_t = ptp.tile([P, QLEN], bf16, tag="pt",
                                    name=f"pt{hp}_{j}_{hb}")
                    nc.scalar.activation(pt_t, sc, AF.Exp,
                                         bias=mask_sb[:, j:j + 1], scale=SCALE)
                    cur_pt.append(pt_t)

                if prev_pt is not None:
                    emit_pv(nc, v_sb, ones64, av, den, hp, j - 1, prev_pt)
                prev_pt = cur_pt

                if j in proj_work:
                    kind, sl0 = proj_work[j]
                    borrow = scp.tile([P, QLEN], f32, tag="sc",
                                      name=f"bw{hp}_{j}")
                    if kind == "k":
                        if sl0 == 0:
                            wc_k = load_wcol(wk, hp + 1, "wkc")
                        for g in range(2):
                            sl = sl0 + g
                            kq_group(borrow[:, g * NSL:(g + 1) * NSL],
                                     wc_k, ht_sb, sl)
                            nc.vector.tensor_copy(
                                kt_sb[hp + 1][:, sl * NSL:(sl + 1) * NSL],
                                borrow[:, g * NSL:(g + 1) * NSL])
                    else:
                        wc_q = load_wcol(wq, hp + 1, "wqc")
                        for g in range(IS):
                            kq_group(borrow[:, g * NSL:(g + 1) * NSL],
                                     wc_q, htq_sb, g)
                            nc.vector.tensor_copy(
                                qt_sb[hp + 1][:, g * NSL:(g + 1) * NSL],
                                borrow[:, g * NSL:(g + 1) * NSL])

            # last PV round
            emit_pv(nc, v_sb, ones64, av, den, hp, JT - 1, prev_pt)

            # evacuate psum promptly (frees banks for the next head pair),
            # then normalize off the critical path: den rows are the
            # denominator already replicated across 64 partitions per head.
            for i in range(IS):
                avc = nrm.tile([P, NSL], f32, tag="avc", name=f"avc{hp}_{i}")
                nc.vector.tensor_copy(avc, av[i])
                dnc = nrm.tile([P, NSL], f32, tag="dnc", name=f"dnc{hp}_{i}")
                nc.vector.tensor_copy(dnc, den[i])
                rep = nrm.tile([P, NSL], f32, tag="rep", name=f"rep{hp}_{i}")
                nc.vector.reciprocal(rep, dnc)
                for hb in range(2):
                    nc.vector.scalar_tensor_tensor(
                        out=avt_sb[hp][hb * 64:(hb + 1) * 64,
                                       i * NSL:(i + 1) * NSL],
                        in0=avc[hb * 64:(hb + 1) * 64, :], scalar=1.0,
                        in1=rep[hb * 64:(hb + 1) * 64, :],
                        op0=ALU.mult, op1=ALU.mult)

        # ---- output projection + residual + layernorm -----------------------
        attn_ctx.close()
        ph1_ctx.close()

        pso = ctx.enter_context(tc.tile_pool(name="pso", bufs=4, space="PSUM"))
        lnp = ctx.enter_context(tc.tile_pool(name="lnp", bufs=2))
        lns = ctx.enter_context(tc.tile_pool(name="lns", bufs=8))

        for t in range(TQ):
            hq_t = lnp.tile([P, D_MODEL], f32, tag="hq", name=f"hq{t}")
            nc.sync.dma_start(out=hq_t, in_=hq[t * P:(t + 1) * P, :])
            xs = lnp.tile([P, D_MODEL], f32, tag="xs", name=f"xs{t}")
            sums = lns.tile([P, 2], f32, tag="sm", name=f"sm{t}")
            for m in range(2):
                ps = pso.tile([P, NSL], f32, tag="po", name=f"po{t}_{m}")
                for e in range(ET):
                    nc.tensor.matmul(ps, avt_sb[e][:, t * P:(t + 1) * P],
                                     wo_sb[e][:, m * NSL:(m + 1) * NSL],
                                     start=(e == 0), stop=(e == ET - 1))
                nc.vector.scalar_tensor_tensor(
                    out=xs[:, m * NSL:(m + 1) * NSL], in0=ps, scalar=1.0,
                    in1=hq_t[:, m * NSL:(m + 1) * NSL],
                    op0=ALU.mult, op1=ALU.add,
                    accum_out=sums[:, m:m + 1])
            # mean/var via accum sums + ACT Square pass (keeps the tail off
            # the DVE): mean = (s0+s1)/D; var = sq/D - mean^2
            sq = lns.tile([P, 2], f32, tag="sq", name=f"sq{t}")
            xsq = lnp.tile([P, D_MODEL], f32, tag="xq", name=f"xq{t}")
            for m in range(2):
                nc.scalar.activation(xsq[:, m * NSL:(m + 1) * NSL],
                                     xs[:, m * NSL:(m + 1) * NSL], AF.Square,
                                     accum_out=sq[:, m:m + 1])
            mean = lns.tile([P, 1], f32, tag="mn", name=f"mn{t}")
            nc.vector.tensor_add(mean, sums[:, 0:1], sums[:, 1:2])
            nc.vector.tensor_scalar_mul(mean, mean, 1.0 / D_MODEL)
            msq = lns.tile([P, 1], f32, tag="mq", name=f"mq{t}")
            nc.vector.tensor_mul(msq, mean, mean)
            var = lns.tile([P, 1], f32, tag="vr", name=f"vr{t}")
            nc.vector.tensor_add(var, sq[:, 0:1], sq[:, 1:2])
            nc.vector.scalar_tensor_tensor(
                out=var, in0=var, scalar=1.0 / D_MODEL, in1=msq,
                op0=ALU.mult, op1=ALU.subtract)
            std = lns.tile([P, 1], f32, tag="sd", name=f"sd{t}")
            nc.scalar.activation(std, var, AF.Sqrt, bias=eps_sb[:, 0:1])
            rstd = lns.tile([P, 1], f32, tag="rs", name=f"rs{t}")
            nc.vector.reciprocal(rstd, std)
            nmr = lns.tile([P, 1], f32, tag="nm", name=f"nm{t}")
            nc.vector.tensor_scalar_mul(nmr, mean, -1.0)
            gs = lnp.tile([P, D_MODEL], f32, tag="gs", name=f"gs{t}")
            nc.vector.tensor_scalar(out=gs, in0=gam_sb,
                                    scalar1=rstd[:, 0:1], scalar2=None,
                                    op0=ALU.mult)
            xg = lnp.tile([P, D_MODEL], f32, tag="xg", name=f"xg{t}")
            nc.vector.scalar_tensor_tensor(
                out=xg, in0=xs, scalar=nmr[:, 0:1], in1=gs,
                op0=ALU.add, op1=ALU.mult)
            xn = lnp.tile([P, D_MODEL], f32, tag="xn", name=f"xn{t}")
            nc.gpsimd.tensor_add(xn, xg, bet_sb)
            nc.sync.dma_start(out=out[t * P:(t + 1) * P, :], in_=xn)

    nc.compile()
    return nc


def _get_nc():
    if "nc" not in _CACHE:
        _CACHE["nc"] = _build()
    return _CACHE["nc"]


def _make_in_maps(inputs):
    bf = ml_dtypes.bfloat16
    h = np.asarray(inputs["h"], dtype=np.float32)
    mask = np.asarray(inputs["attn_mask"])
    Wq = np.asarray(inputs["Wq"], dtype=np.float32)
    Wkv = np.asarray(inputs["Wkv"], dtype=np.float32)
    Wo = np.asarray(inputs["Wo"], dtype=np.float32)
    gamma = np.asarray(inputs["gamma"], dtype=np.float32)
    beta = np.asarray(inputs["beta"], dtype=np.float32)

    wq_b = np.ascontiguousarray(Wq.astype(bf))
    wk_b = np.ascontiguousarray(Wkv[:, :D_MODEL].astype(bf))
    wv_b = np.ascontiguousarray(Wkv[:, D_MODEL:].astype(bf))
    wo_b = np.ascontiguousarray(Wo.astype(bf))

    in_maps = []
    for c in range(8):
        b, half = divmod(c, 2)
        hb = h[:, b, :]
        hT_b = np.ascontiguousarray(hb.T.astype(bf))
        in_maps.append({
            "hT": hT_b,
            "hTq": np.ascontiguousarray(hT_b[:, half * QLEN:(half + 1) * QLEN]),
            "hq": np.ascontiguousarray(hb[half * QLEN:(half + 1) * QLEN, :]),
            "wq": wq_b, "wk": wk_b, "wv": wv_b, "wo": wo_b,
            "mb": np.where(mask[:, b], np.float32(-1e9), np.float32(0.0)),
            "gam": gamma, "bet": beta,
        })
    return in_maps


def _run(in_maps, **kwargs):
    from concourse.bass_utils import run_bass_kernel_spmd
    return run_bass_kernel_spmd(_get_nc(), in_maps, core_ids=list(range(8)),
                                **kwargs)


def kernel(**inputs) -> np.ndarray:
    res = _run(_make_in_maps(inputs))
    out = np.empty((SEQ, BSZ, D_MODEL), dtype=np.float32)
    for c in range(8):
        b, half = divmod(c, 2)
        out[half * QLEN:(half + 1) * QLEN, :, :][:, b, :] = res.results[c]["out"]
    return out


# revision 18
# speedup vs baseline: 1.0231x; 1.0231x over previous
"""MultiHeadAttn (post-LN, key-padding mask) Trainium2 Bass kernel, 8 cores.

Problem: h [S=2048, B=4, D=1024] f32; 16 heads x 64; key-padding mask [S, B];
out = LayerNorm(h + MHA(h)).

Sharding: core c handles batch b = c//2 and query half qh = c%2 (1024 query
rows), with all 16 heads and the full 2048-key context for that batch.
KV projections are recomputed by both cores of a batch pair (no collectives).

Per-core device pipeline (all matmuls bf16, fp32 accumulation in PSUM):
  - K^T/Q^T proj: stationary Wk/Wq column tiles, moving h^T -> [e, t] layout.
  - V proj: stationary h^T tiles, moving Wv -> natural [t, e] layout, stored
    with an appended ones column per head (gives softmax denominators via PV).
  - Attention per head pair: scores^T [j,i] via row-paired matmuls (two heads
    in row strips 0-63 / 64-127 of the PE array), exp via ScalarE with the
    key-padding bias as a per-partition bias and 1/sqrt(dh) as the scale,
    PV with ones-augmented V (M=65; row 64 accumulates the denominator),
    then normalize via reciprocal + partition broadcast + multiply.
  - Output proj: stationary attn_vec^T tiles, moving Wo; residual add + LN
    (bn_stats/bn_aggr) fused on DVE/ScalarE.
Next head pair's K/Q projections are interleaved into the attention loop
(borrowing scores-pool PSUM slots) so the PE stays busy under the ACT-bound
softmax stream.
"""
import numpy as np
import ml_dtypes

N_HEAD, D_MODEL, D_HEAD = 16, 1024, 64
SEQ, BSZ = 2048, 4
QLEN = SEQ // 2
SCALE = 1.0 / D_HEAD ** 0.5
LN_EPS = 1e-5
P = 128
NSL = 512                   # matmul moving-operand slab (one PSUM bank fp32)
CT = D_MODEL // P           # 8 contraction tiles
ET = D_MODEL // P           # 8 e-tiles (2 heads each)
JT = SEQ // P               # 16 key tiles
JS = SEQ // NSL             # 4 key slabs
IS = QLEN // NSL            # 2 query slabs
TQ = QLEN // P              # 8 query-row tiles
HP = N_HEAD // 2            # 8 head pairs

_CACHE = {}


def _build():
    from contextlib import ExitStack
    import concourse.bass as bass
    import concourse.mybir as mybir
    import concourse.tile as tile
    from concourse import bacc

    dt = mybir.dt
    f32, bf16 = dt.float32, dt.bfloat16
    AF = mybir.ActivationFunctionType
    ALU = mybir.AluOpType

    nc = bacc.Bacc(None, target_bir_lowering=False)

    hT = nc.dram_tensor("hT", [D_MODEL, SEQ], bf16, kind="ExternalInput")
    hq = nc.dram_tensor("hq", [QLEN, D_MODEL], f32, kind="ExternalInput")
    wq = nc.dram_tensor("wq", [D_MODEL, D_MODEL], bf16, kind="ExternalInput")
    wk = nc.dram_tensor("wk", [D_MODEL, D_MODEL], bf16, kind="ExternalInput")
    wv = nc.dram_tensor("wv", [D_MODEL, D_MODEL], bf16, kind="ExternalInput")
    wo = nc.dram_tensor("wo", [D_MODEL, D_MODEL], bf16, kind="ExternalInput")
    mb = nc.dram_tensor("mb", [SEQ], f32, kind="ExternalInput")
    gam = nc.dram_tensor("gam", [D_MODEL], f32, kind="ExternalInput")
    bet = nc.dram_tensor("bet", [D_MODEL], f32, kind="ExternalInput")
    out = nc.dram_tensor("out", [QLEN, D_MODEL], f32, kind="ExternalOutput")

    with tile.TileContext(nc) as tc, ExitStack() as ctx:
        persist = ctx.enter_context(tc.tile_pool(name="persist", bufs=1))

        kt_sb = [persist.tile([P, SEQ], bf16, name=f"kt{e}") for e in range(ET)]
        qt_sb = [persist.tile([P, QLEN], bf16, name=f"qt{e}") for e in range(ET)]
        v_sb = [persist.tile([P, N_HEAD, D_HEAD], bf16, name=f"v{t}")
                for t in range(JT)]
        ones64 = persist.tile([P, 64], bf16, name="ones64")
        avt_sb = [persist.tile([P, QLEN], bf16, name=f"avt{e}") for e in range(ET)]
        mask_sb = persist.tile([P, JT], f32, name="mask")
        eps_sb = persist.tile([P, 1], f32, name="eps")

        nc.vector.memset(eps_sb, LN_EPS)
        nc.vector.memset(ones64, 1.0)

        nc.gpsimd.dma_start(out=mask_sb,
                            in_=bass.AP(tensor=mb, offset=0, ap=[[1, P], [P, JT]]))

        # ---- phase-3 weights: load early into the region wvp freed ---------
        w3p = ctx.enter_context(tc.tile_pool(name="w3p", bufs=1))
        wo_sb = [w3p.tile([P, D_MODEL], bf16, name=f"wo{c}") for c in range(CT)]
        gam_sb = w3p.tile([P, D_MODEL], f32, name="gamr")
        bet_sb = w3p.tile([P, D_MODEL], f32, name="betr")

        # ---- phase 1 scope: h^T residency + streamed W columns --------------
        ph1_ctx = ExitStack()
        ph1 = ph1_ctx.enter_context(tc.tile_pool(name="ph1", bufs=1))
        ht_sb = [ph1.tile([P, SEQ], bf16, name=f"ht{c}") for c in range(CT)]

        wcol = ph1_ctx.enter_context(tc.tile_pool(name="wcol", bufs=3))

        def load_wcol(w, e, tag):
            wc = wcol.tile([P, CT, P], bf16, tag=tag, name=f"{tag}{e}")
            nc.sync.dma_start(
                out=wc,
                in_=w[:, e * P:(e + 1) * P].rearrange("(ct p) e -> p ct e", p=P))
            return wc

        # startup DMA priority: the first K-proj matmul needs wkc(0) + ht
        # tiles, so those go first on the sync queue; htq (needed later, for
        # Q-proj) goes via gpsimd SWDGE in parallel.
        wc0 = load_wcol(wk, 0, "wkc")
        for c in range(CT):
            eng = nc.sync if c % 2 == 0 else nc.scalar
            eng.dma_start(out=ht_sb[c], in_=hT[c * P:(c + 1) * P, :])

        def kq_group(ps_ap, wc, moving, sl):
            """8 accumulating matmuls: one K/Q-proj output group into psum."""
            for c in range(CT):
                nc.tensor.matmul(ps_ap, wc[:, c, :],
                                 moving[c][:, sl * NSL:(sl + 1) * NSL],
                                 start=(c == 0), stop=(c == CT - 1))

        # prephase: K(0), Q(0), V (own pools, closed before attention)
        with tc.tile_pool(name="wvp", bufs=1) as wvp, \
             tc.tile_pool(name="psA", bufs=6, space="PSUM") as psA:
            wv_sb = [wvp.tile([P, D_MODEL], bf16, name=f"wv{c}") for c in range(CT)]
            for c in range(CT):
                nc.scalar.dma_start(out=wv_sb[c], in_=wv[c * P:(c + 1) * P, :])
            wc = wc0
            for j in range(JS):
                ps = psA.tile([P, NSL], f32, tag="psa", name=f"psk0_{j}")
                kq_group(ps, wc, ht_sb, j)
                nc.vector.tensor_copy(kt_sb[0][:, j * NSL:(j + 1) * NSL], ps)
            wc = load_wcol(wq, 0, "wqc")
            for i in range(IS):
                ps = psA.tile([P, NSL], f32, tag="psa", name=f"psq0_{i}")
                kq_group(ps, wc, ht_sb, i)
                nc.vector.tensor_copy(qt_sb[0][:, i * NSL:(i + 1) * NSL], ps)
            # V projection: stationary h^T tiles, moving Wv slabs
            for t in range(JT):
                for es in range(2):
                    ps = psA.tile([P, NSL], f32, tag="psa", name=f"psv{t}_{es}")
                    for c in range(CT):
                        nc.tensor.matmul(ps, ht_sb[c][:, t * P:(t + 1) * P],
                                         wv_sb[c][:, es * NSL:(es + 1) * NSL],
                                         start=(c == 0), stop=(c == CT - 1))
                    nc.vector.tensor_copy(
                        v_sb[t][:, es * 8:(es + 1) * 8, :],
                        ps[:, :].rearrange("p (h d) -> p h d", d=D_HEAD))

        def emit_pv(nc, v_sb, ones64, av, den, hp, j, pts):
            first, last = (j == 0), (j == JT - 1)
            for i in range(IS):
                sl = slice(i * NSL, (i + 1) * NSL)
                nc.tensor.matmul(av[i][0:64, :], v_sb[j][:, hp * 2, :],
                                 pts[0][:, sl], start=first, stop=last,
                                 tile_position=(0, 0))
                nc.tensor.matmul(av[i][64:P, :], v_sb[j][:, hp * 2 + 1, :],
                                 pts[1][:, sl], start=first, stop=last,
                                 tile_position=(0, 64), skip_group_check=True)
                nc.tensor.matmul(den[i][0:64, :], ones64, pts[0][:, sl],
                                 start=first, stop=last,
                                 tile_position=(0, 0), skip_group_check=True)
                nc.tensor.matmul(den[i][64:P, :], ones64, pts[1][:, sl],
                                 start=first, stop=last,
                                 tile_position=(0, 64), skip_group_check=True)

        for c in range(CT):
            nc.scalar.dma_start(out=wo_sb[c], in_=wo[c * P:(c + 1) * P, :])
        nc.gpsimd.dma_start(out=gam_sb,
                            in_=bass.AP(tensor=gam, offset=0, ap=[[0, P], [1, D_MODEL]]))
        nc.gpsimd.dma_start(out=bet_sb,
                            in_=bass.AP(tensor=bet, offset=0, ap=[[0, P], [1, D_MODEL]]))

        # ---- attention ------------------------------------------------------
        attn_ctx = ExitStack()
        scp = attn_ctx.enter_context(tc.tile_pool(name="scp", bufs=2, space="PSUM"))
        avp = attn_ctx.enter_context(tc.tile_pool(name="avp", bufs=2, space="PSUM"))
        ptp = attn_ctx.enter_context(tc.tile_pool(name="ptp", bufs=8))
        nrm = attn_ctx.enter_context(tc.tile_pool(name="nrm", bufs=3))

        for hp in range(HP):
            av = [avp.tile([P, NSL], f32, tag="av", name=f"av{hp}_{i}")
                  for i in range(IS)]
            den = [avp.tile([P, NSL], f32, tag="den", name=f"den{hp}_{i}")
                   for i in range(IS)]
            # interleaved projection work for the NEXT head pair, borrowing
            # scores-pool psum slots: (emit_at_j, which, slab)
            proj_work = {4: ("k", 0), 8: ("k", 2), 12: ("q", 0)} if hp + 1 < HP else {}
            prev_pt = None
            wc_k = None

            for j in range(JT):
                cur_pt = []
                for hb in range(2):
                    h = hp * 2 + hb
                    base = hb * 64
                    sc = scp.tile([P, QLEN], f32, tag="sc", name=f"sc{hp}_{j}_{hb}")
                    for i in range(IS):
                        nc.tensor.matmul(
                            sc[:, i * NSL:(i + 1) * NSL],
                            kt_sb[hp][base:base + 64, j * P:(j + 1) * P],
                            qt_sb[hp][base:base + 64, i * NSL:(i + 1) * NSL],
                            start=True, stop=True, tile_position=(base, 0))
                    pt_t = ptp.tile([P, QLEN], bf16, tag="pt",
                                    name=f"pt{hp}_{j}_{hb}")
                    nc.scalar.activation(pt_t, sc, AF.Exp,
                                         bias=mask_sb[:, j:j + 1], scale=SCALE)
                    cur_pt.append(pt_t)

                if prev_pt is not None:
                    emit_pv(nc, v_sb, ones64, av, den, hp, j - 1, prev_pt)
                prev_pt = cur_pt

                if j in proj_work:
                    kind, sl0 = proj_work[j]
                    borrow = scp.tile([P, QLEN], f32, tag="sc",
                                      name=f"bw{hp}_{j}")
                    if kind == "k":
                        if sl0 == 0:
                            wc_k = load_wcol(wk, hp + 1, "wkc")
                        for g in range(2):
                            sl = sl0 + g
                            kq_group(borrow[:, g * NSL:(g + 1) * NSL],
                                     wc_k, ht_sb, sl)
                            nc.vector.tensor_copy(
                                kt_sb[hp + 1][:, sl * NSL:(sl + 1) * NSL],
                                borrow[:, g * NSL:(g + 1) * NSL])
                    else:
                        wc_q = load_wcol(wq, hp + 1, "wqc")
                        for g in range(IS):
                            kq_group(borrow[:, g * NSL:(g + 1) * NSL],
                                     wc_q, ht_sb, g)
                            nc.vector.tensor_copy(
                                qt_sb[hp + 1][:, g * NSL:(g + 1) * NSL],
                                borrow[:, g * NSL:(g + 1) * NSL])

            # last PV round
            emit_pv(nc, v_sb, ones64, av, den, hp, JT - 1, prev_pt)

            # evacuate psum promptly (frees banks for the next head pair),
            # then normalize off the critical path: den rows are the
            # denominator already replicated across 64 partitions per head.
            for i in range(IS):
                avc = nrm.tile([P, NSL], f32, tag="avc", name=f"avc{hp}_{i}")
                nc.vector.tensor_copy(avc, av[i])
                dnc = nrm.tile([P, NSL], f32, tag="dnc", name=f"dnc{hp}_{i}")
                nc.vector.tensor_copy(dnc, den[i])
                rep = nrm.tile([P, NSL], f32, tag="rep", name=f"rep{hp}_{i}")
                nc.vector.reciprocal(rep, dnc)
                for hb in range(2):
                    nc.vector.scalar_tensor_tensor(
                        out=avt_sb[hp][hb * 64:(hb + 1) * 64,
                                       i * NSL:(i + 1) * NSL],
                        in0=avc[hb * 64:(hb + 1) * 64, :], scalar=1.0,
                        in1=rep[hb * 64:(hb + 1) * 64, :],
                        op0=ALU.mult, op1=ALU.mult)

        # ---- output projection + residual + layernorm -----------------------
        attn_ctx.close()
        ph1_ctx.close()

        pso = ctx.enter_context(tc.tile_pool(name="pso", bufs=4, space="PSUM"))
        lnp = ctx.enter_context(tc.tile_pool(name="lnp", bufs=2))
        lns = ctx.enter_context(tc.tile_pool(name="lns", bufs=8))

        for t in range(TQ):
            hq_t = lnp.tile([P, D_MODEL], f32, tag="hq", name=f"hq{t}")
            nc.sync.dma_start(out=hq_t, in_=hq[t * P:(t + 1) * P, :])
            xs = lnp.tile([P, D_MODEL], f32, tag="xs", name=f"xs{t}")
            sums = lns.tile([P, 2], f32, tag="sm", name=f"sm{t}")
            for m in range(2):
                ps = pso.tile([P, NSL], f32, tag="po", name=f"po{t}_{m}")
                for e in range(ET):
                    nc.tensor.matmul(ps, avt_sb[e][:, t * P:(t + 1) * P],
                                     wo_sb[e][:, m * NSL:(m + 1) * NSL],
                                     start=(e == 0), stop=(e == ET - 1))
                nc.vector.scalar_tensor_tensor(
                    out=xs[:, m * NSL:(m + 1) * NSL], in0=ps, scalar=1.0,
                    in1=hq_t[:, m * NSL:(m + 1) * NSL],
                    op0=ALU.mult, op1=ALU.add,
                    accum_out=sums[:, m:m + 1])
            # mean/var via accum sums + ACT Square pass (keeps the tail off
            # the DVE): mean = (s0+s1)/D; var = sq/D - mean^2
            sq = lns.tile([P, 2], f32, tag="sq", name=f"sq{t}")
            xsq = lnp.tile([P, D_MODEL], f32, tag="xq", name=f"xq{t}")
            for m in range(2):
                nc.scalar.activation(xsq[:, m * NSL:(m + 1) * NSL],
                                     xs[:, m * NSL:(m + 1) * NSL], AF.Square,
                                     accum_out=sq[:, m:m + 1])
            mean = lns.tile([P, 1], f32, tag="mn", name=f"mn{t}")
            nc.vector.tensor_add(mean, sums[:, 0:1], sums[:, 1:2])
            nc.vector.tensor_scalar_mul(mean, mean, 1.0 / D_MODEL)
            msq = lns.tile([P, 1], f32, tag="mq", name=f"mq{t}")
            nc.vector.tensor_mul(msq, mean, mean)
            var = lns.tile([P, 1], f32, tag="vr", name=f"vr{t}")
            nc.vector.tensor_add(var, sq[:, 0:1], sq[:, 1:2])
            nc.vector.scalar_tensor_tensor(
                out=var, in0=var, scalar=1.0 / D_MODEL, in1=msq,
                op0=ALU.mult, op1=ALU.subtract)
            std = lns.tile([P, 1], f32, tag="sd", name=f"sd{t}")
            nc.scalar.activation(std, var, AF.Sqrt, bias=eps_sb[:, 0:1])
            rstd = lns.tile([P, 1], f32, tag="rs", name=f"rs{t}")
            nc.vector.reciprocal(rstd, std)
            nmr = lns.tile([P, 1], f32, tag="nm", name=f"nm{t}")
            nc.vector.tensor_scalar_mul(nmr, mean, -1.0)
            gs = lnp.tile([P, D_MODEL], f32, tag="gs", name=f"gs{t}")
            nc.vector.tensor_scalar(out=gs, in0=gam_sb,
                                    scalar1=rstd[:, 0:1], scalar2=None,
                                    op0=ALU.mult)
            xg = lnp.tile([P, D_MODEL], f32, tag="xg", name=f"xg{t}")
            nc.vector.scalar_tensor_tensor(
                out=xg, in0=xs, scalar=nmr[:, 0:1], in1=gs,
                op0=ALU.add, op1=ALU.mult)
            xn = lnp.tile([P, D_MODEL], f32, tag="xn", name=f"xn{t}")
            nc.gpsimd.tensor_add(xn, xg, bet_sb)
            nc.sync.dma_start(out=out[t * P:(t + 1) * P, :], in_=xn)

    nc.compile()
    return nc


def _get_nc():
    if "nc" not in _CACHE:
        _CACHE["nc"] = _build()
    return _CACHE["nc"]


def _make_in_maps(inputs):
    bf = ml_dtypes.bfloat16
    h = np.asarray(inputs["h"], dtype=np.float32)
    mask = np.asarray(inputs["attn_mask"])
    Wq = np.asarray(inputs["Wq"], dtype=np.float32)
    Wkv = np.asarray(inputs["Wkv"], dtype=np.float32)
    Wo = np.asarray(inputs["Wo"], dtype=np.float32)
    gamma = np.asarray(inputs["gamma"], dtype=np.float32)
    beta = np.asarray(inputs["beta"], dtype=np.float32)

    wq_b = np.ascontiguousarray(Wq.astype(bf))
    wk_b = np.ascontiguousarray(Wkv[:, :D_MODEL].astype(bf))
    wv_b = np.ascontiguousarray(Wkv[:, D_MODEL:].astype(bf))
    wo_b = np.ascontiguousarray(Wo.astype(bf))

    in_maps = []
    for c in range(8):
        b, half = divmod(c, 2)
        hb = h[:, b, :]
        hT_b = hb.T.astype(bf)
        own = slice(half * QLEN, (half + 1) * QLEN)
        other = slice((1 - half) * QLEN, (2 - half) * QLEN)
        # own query-half first: keys are in core-local order, so the Q
        # projection can read the first half of hT uniformly on every core.
        # The mask is reordered identically; attention is key-order-invariant.
        hT_r = np.ascontiguousarray(np.concatenate(
            [hT_b[:, own], hT_b[:, other]], axis=1))
        mb_full = np.where(mask[:, b], np.float32(-1e9), np.float32(0.0))
        in_maps.append({
            "hT": hT_r,
            "hq": np.ascontiguousarray(hb[own, :]),
            "wq": wq_b, "wk": wk_b, "wv": wv_b, "wo": wo_b,
            "mb": np.ascontiguousarray(
                np.concatenate([mb_full[own], mb_full[other]])),
            "gam": gamma, "bet": beta,
        })
    return in_maps


def _run(in_maps, **kwargs):
    from concourse.bass_utils import run_bass_kernel_spmd
    return run_bass_kernel_spmd(_get_nc(), in_maps, core_ids=list(range(8)),
                                **kwargs)


def kernel(**inputs) -> np.ndarray:
    res = _run(_make_in_maps(inputs))
    out = np.empty((SEQ, BSZ, D_MODEL), dtype=np.float32)
    for c in range(8):
        b, half = divmod(c, 2)
        out[half * QLEN:(half + 1) * QLEN, :, :][:, b, :] = res.results[c]["out"]
    return out
